# revision 41
# baseline (speedup 1.0000x reference)
"""MemN2N (3-hop memory network) forward pass on 8 Trainium2 NeuronCores.

Strategy (per spec sharding hint):
 - Data-parallel over batch (4 batches/core) for the embedding gathers and
   hop math. Embedding tables are replicated in each core's DRAM, merged
   into one [V, 1536B] row = [A1 bf16 | A2,A3,A4 fp8-e4m3 | pad]: one
   gather fetches all 4 tables per word. A1 must stay bf16 (the hop-1
   score path is precision-sensitive: fp8 A1 gives 3e-1 max-rel error vs
   9.6e-3 with this split; gate is 2e-2). Rows are padded 1280->1536B so
   every gather element is 512B-aligned (~25% faster gathers measured).
   The A2..A4 word-contractions run as fp8 x fp8 matmuls (bf16-mixed
   operands hit a 5-6x slower PE path on HW); the position-encoding
   weight column a_l = l-15.5 is split hi=trunc(a), lo=a-hi, both
   e4m3-exact, accumulated over two matmul passes in PSUM.
 - Position encoding pe[l,e] = 1 + a_l*b_e is rank-2, so each embedding
   reduction over words L needs only two weighted sums (plain and a-weighted),
   computed on the TensorEngine with a block-diagonal weight matrix
   (K = 4 batches x 32 word-slots). Results land in PSUM as
   [e-half partitions, (q, t, h, r*2+s)] and are flushed to SBUF in one
   contiguous copy per chunk on the vector queue.
 - After the hops, u3 (4,256) is AllGathered; each core then computes the
   final linear for its 4000-vocab shard, BatchNorm over the full batch,
   and a log-softmax whose log-sum-exp is combined across cores via a second
   tiny AllGather of per-core per-batch sum-exp values.
 - The repeat loop is software-pipelined: iteration k's gathers/emb phase is
   emitted interleaved with iteration k-1's hops/head/collectives so the DMA
   engines (the ~74us/iter roofline) never idle during the serial tail.

kernel(**inputs) takes the FULL unsharded inputs and returns the FULL
(32, 32000) float32 output.
"""
import sys
sys.path.insert(0, '/opt/trn_rl_repo')
import numpy as np
import ml_dtypes
from contextlib import ExitStack

import concourse.bass as bass
import concourse.bacc as bacc
import concourse.tile as tile
from concourse import mybir
from concourse.alu_op_type import AluOpType
from concourse.bass_utils import run_bass_kernel_spmd

F32 = mybir.dt.float32
BF16 = mybir.dt.bfloat16
F8 = mybir.dt.float8e4
I16 = mybir.dt.int16
AF = mybir.ActivationFunctionType
AX = mybir.AxisListType

# fp8 e4m3 numpy dtype matching mybir.dt.float8e4 on device
FP8NP = ml_dtypes.float8_e4m3

# problem constants
B, M, L, LQ, E, V = 32, 100, 30, 30, 256, 32000
NC_ = 8
BL = B // NC_          # 4 local batches
VL = V // NC_          # 4000 local vocab
CH = 128               # head vocab-chunk size (31 full + 1 of 32 = 4000)
NCH = 32               # chunks (last chunk only 32 rows)
NQ = 8                 # gather chunk: 8 m-columns -> 1024 indices
EH = E // 2            # 128, e-half
NCHUNK = (M + NQ - 1) // NQ   # 13

_cache = {}

# PE contraction mode for the fp8 tables (A2..A4):
#   'mixed' : lhsT fp8, rhs = bf16 wmat (single pass)
#   'fp8_2p': lhsT fp8, rhs = fp8 wmat split hi/lo (2 accumulating passes;
#             exact because a_l = int + 0.5 and both parts are e4m3-exact)
MM_MODE = 'fp8_2p'
# merged table row: [A1 bf16 512B | A2|A3|A4 fp8 768B | pad to ROW_B]
ROW_B = 1536
ROW_W = ROW_B // 2     # bf16 units


def _consts():
    a = (np.arange(1, L + 1, dtype=np.float64) - (L + 1) / 2.0)        # (30,)
    b = 4.0 * (np.arange(1, E + 1, dtype=np.float64) - (E + 1) / 2.0) / (E * L)
    wmat = np.zeros((128, 8), np.float32)
    for r in range(4):
        for l in range(L):
            wmat[32 * r + l, 2 * r + 0] = 1.0
            wmat[32 * r + l, 2 * r + 1] = a[l]
    # e4m3-exact hi/lo split of wmat: hi = [1-col, trunc(a)], lo = [0, a-hi]
    wm_hi = np.zeros((128, 8), np.float32)
    wm_lo = np.zeros((128, 8), np.float32)
    for r in range(4):
        for l in range(L):
            wm_hi[32 * r + l, 2 * r + 0] = 1.0
            wm_hi[32 * r + l, 2 * r + 1] = np.trunc(a[l])
            wm_lo[32 * r + l, 2 * r + 1] = a[l] - np.trunc(a[l])
    bvec = b.astype(np.float32).reshape(2, EH).T.copy()               # [128,2]
    ones1 = np.ones((1, 128), np.float32)
    ident = np.eye(128, dtype=np.float32)
    return wmat, wm_hi, wm_lo, bvec, ones1, ident


def _wrap_idx(flat):
    """int16 flat index array (len % 16 == 0) -> [128, len//16] wrapped+tiled."""
    n = flat.shape[0]
    wr = flat.reshape(n // 16, 16).T.astype(np.int16)   # [16, n//16]
    return np.tile(wr, (8, 1)).copy()                   # [128, n//16]


def build_nc(repeat=1, sim_no_coll=False, dump=False, mm_mode=None):
    mm_mode = mm_mode or MM_MODE
    nc = bacc.Bacc("TRN2", target_bir_lowering=False, debug=False,
                   num_devices=NC_, dynamic_dma_scratch_size=65536)

    # ---- DRAM I/O ----
    # Merged table row = [A1 bf16 512B | A2 fp8 256B | A3 fp8 256B | A4 fp8
    # 256B | pad] = ROW_B (512-aligned rows measure ~30% faster to gather
    # than packed-1280B). A1 stays bf16 (hop-1 score side is precision-
    # sensitive); A2..A4 as fp8 e4m3 (measured end-to-end 9.6e-3 < 2e-2).
    # One gather per chunk keeps SWDGE descriptor-gen at the v1 rate.
    tabm = nc.dram_tensor("tabm", [V, ROW_W], BF16, kind="ExternalInput").ap()
    sidx = nc.dram_tensor("sidx", [128, M * 128 // 16], I16, kind="ExternalInput").ap()
    qidx = nc.dram_tensor("qidx", [128, 8], I16, kind="ExternalInput").ap()
    qmc = nc.dram_tensor("qmc", [128, 1], F32, kind="ExternalInput").ap()
    pmf = nc.dram_tensor("pmf", [128, M], F32, kind="ExternalInput").ap()
    wti = nc.dram_tensor("wti", [2, 128, VL], BF16, kind="ExternalInput").ap()
    gbi = nc.dram_tensor("gbi", [128, 2, NCH], F32, kind="ExternalInput").ap()
    vmi = nc.dram_tensor("vmi", [128, NCH, B], F32, kind="ExternalInput").ap()
    wmi = nc.dram_tensor("wmi", [128, 8], BF16, kind="ExternalInput").ap()
    wmh8i = nc.dram_tensor("wmh8i", [128, 8], F8, kind="ExternalInput").ap()
    wml8i = nc.dram_tensor("wml8i", [128, 8], F8, kind="ExternalInput").ap()
    bvi = nc.dram_tensor("bvi", [128, 2], F32, kind="ExternalInput").ap()
    on1 = nc.dram_tensor("on1", [1, 128], F32, kind="ExternalInput").ap()
    idi = nc.dram_tensor("idi", [128, 128], F32, kind="ExternalInput").ap()
    out = nc.dram_tensor("out", [128, NCH, B], BF16,
                         kind="ExternalOutput").ap()

    u3_loc = nc.dram_tensor("u3_loc", [2, 128, BL], F32).ap()
    u3_gth = nc.dram_tensor("u3_gth", [NC_, 2, 128, BL], F32,
                            addr_space="Shared").ap()
    lse_loc = nc.dram_tensor("lse_loc", [B], F32).ap()
    lse_gth = nc.dram_tensor("lse_gth", [NC_, B], F32,
                             addr_space="Shared").ap()

    with tile.TileContext(nc) as tc, ExitStack() as ctx:
        cons = ctx.enter_context(tc.tile_pool(name="cons", bufs=1))
        embp = ctx.enter_context(tc.tile_pool(name="embp", bufs=2))
        rt_p = ctx.enter_context(tc.tile_pool(name="rt", bufs=3))
        tmp = ctx.enter_context(tc.tile_pool(name="tmp", bufs=2))
        up = ctx.enter_context(tc.tile_pool(name="up", bufs=2))
        pp_e = ctx.enter_context(tc.tile_pool(name="pp_e", bufs=4, space="PSUM"))
        pp_s = ctx.enter_context(tc.tile_pool(name="pp_s", bufs=1, space="PSUM"))
        pp_w = ctx.enter_context(tc.tile_pool(name="pp_w", bufs=1, space="PSUM"))
        pp_t = ctx.enter_context(tc.tile_pool(name="pp_t", bufs=1, space="PSUM"))

        # ---- constants / small inputs (loaded once) ----
        sidx_sb = cons.tile([128, M * 8], I16)
        nc.sync.dma_start(sidx_sb[:], sidx)
        qidx_sb = cons.tile([128, 8], I16)
        nc.sync.dma_start(qidx_sb[:], qidx)
        wmat = cons.tile([128, 8], BF16)
        nc.sync.dma_start(wmat[:], wmi)
        wmhi8 = cons.tile([128, 8], F8)
        nc.sync.dma_start(wmhi8[:], wmh8i)
        wmlo8 = cons.tile([128, 8], F8)
        nc.sync.dma_start(wmlo8[:], wml8i)
        bvec = cons.tile([128, 2], F32)
        nc.sync.dma_start(bvec[:], bvi)
        ones1 = cons.tile([1, 128], F32)
        nc.sync.dma_start(ones1[:], on1)
        qm_sb = cons.tile([128, 1], F32)
        nc.sync.dma_start(qm_sb[:], qmc)
        ident = cons.tile([128, 128], F32)
        nc.scalar.dma_start(ident[:], idi)
        gb_sb = cons.tile([128, 2, NCH], F32)
        nc.scalar.dma_start(gb_sb[:], gbi)
        wt_sb = cons.tile([128, 2, VL], BF16)
        nc.scalar.dma_start(wt_sb[:, 0, :], wti[0])
        nc.scalar.dma_start(wt_sb[:, 1, :], wti[1])
        logvm = cons.tile([128, NCH, B], F32)
        nc.scalar.dma_start(logvm[:], vmi)

        # loop-invariant derived tiles
        pm4 = cons.tile([128, M], F32)
        nc.sync.dma_start(pm4[:], pmf)
        pm4_m1 = cons.tile([128, M], F32)
        nc.vector.tensor_scalar(pm4_m1[:], pm4[:], -1.0, 1e30,
                                AluOpType.add, AluOpType.mult)
        wmatq = cons.tile([128, 8], BF16)
        nc.vector.tensor_scalar_mul(wmatq[:], wmat[:], qm_sb[:, 0:1])
        # one-hot selector matrices: sel[p, r, q] = (p == 32r), used to
        # replicate softmax row 32r across all 128 partitions via matmul
        sel = cons.tile([128, BL, 128], F32)
        nc.vector.memset(sel[:], 0.0)
        for r in range(BL):
            nc.vector.memset(sel[32 * r:32 * r + 1, r, :], 1.0)
        # scrub the score PSUM bank once: all later writes are finite, so
        # masked-lane arithmetic (0 * stale) can never see boot inf/NaN
        ps_z = pp_s.tile([128, M], F32, tag="scr", bufs=1)
        nc.vector.memset(ps_z[:], 0.0)

        def emb_ap(embt, t, h, r, s, which):
            """AP views of emb_all [128, M, 4, 2, 8] f32; rs index = r*2+s."""
            off = embt[:].offset + t * 16 + h * 8 + (0 if r is None else r * 2) + s
            if which == 'score':       # [128, M] for fixed (t,h,r,s)
                return bass.AP(embt.tensor, off, [embt[:].ap[0], [64, M]])
            if which == 'ored':        # [128, BL, M] for fixed (t,h,s)
                return bass.AP(embt.tensor, off,
                               [embt[:].ap[0], [2, BL], [64, M]])
            raise ValueError(which)

        def emit_query(st):
            """Query encoding -> st['u'] (f32 [128, 2, BL])."""
            rq = rt_p.tile([128, 1, E], BF16, tag="rq")
            tabm_q = bass.AP(tabm.tensor, 0, [[ROW_W, V], [1, E]])
            nc.gpsimd.dma_gather(rq[:], tabm_q, qidx_sb[:, :],
                                 num_idxs=128, num_idxs_reg=128,
                                 elem_size=E, elem_step=ROW_W)
            ps_q = pp_s.tile([128, 16], F32, tag="scr", bufs=1)
            for h in range(2):
                nc.tensor.matmul(ps_q[:, h * 8:(h + 1) * 8],
                                 rq[:, 0, h * EH:(h + 1) * EH],
                                 wmatq[:], start=True, stop=True)
            q_sb = tmp.tile([128, 16], F32, tag="q_sb")
            nc.vector.tensor_copy(q_sb[:], ps_q[:])
            u_cur = up.tile([128, 2, BL], F32, tag="u")
            for h in range(2):
                psq_odd = bass.AP(q_sb.tensor, q_sb[:].offset + h * 8 + 1,
                                  [q_sb[:].ap[0], [2, BL]])
                psq_evn = bass.AP(q_sb.tensor, q_sb[:].offset + h * 8 + 0,
                                  [q_sb[:].ap[0], [2, BL]])
                nc.vector.scalar_tensor_tensor(
                    u_cur[:, h, :], psq_odd, bvec[:, h:h + 1], psq_evn,
                    AluOpType.mult, AluOpType.add)
            st['u'] = u_cur
            st['u0'] = u_cur

        def emit_chunk(st, j):
            """Gather chunk j + PE-reduce; flush is deferred (st['flush'])."""
            q0 = j * NQ
            nq = min(NQ, M - q0)
            rt = rt_p.tile([128, NQ, ROW_W], BF16, tag="rt")
            nc.gpsimd.dma_gather(
                rt[:, :nq, :], tabm,
                sidx_sb[:, q0 * 8:(q0 + nq) * 8],
                num_idxs=nq * 128, num_idxs_reg=nq * 128,
                elem_size=ROW_W, elem_step=ROW_W)
            ps_e = pp_e.tile([128, NQ, 4, 2, 8], F32, tag="pse")
            for q in range(nq):
                for t in range(4):
                    for h in range(2):
                        if t == 0:
                            nc.tensor.matmul(
                                ps_e[:, q, t, h, :],
                                rt[:, q, h * EH:(h + 1) * EH],
                                wmat[:], start=True, stop=True)
                            continue
                        # fp8 bytes live at bf16-element offset
                        # 256 + (t-1)*128 + h*64; 64 bf16 = 128 fp8
                        b0 = 256 + (t - 1) * 128 + h * 64
                        lhsT = rt[:, q, b0:b0 + 64].bitcast(F8)
                        if mm_mode == 'mixed':
                            nc.tensor.matmul(
                                ps_e[:, q, t, h, :], lhsT,
                                wmat[:], start=True, stop=True)
                        else:
                            nc.tensor.matmul(
                                ps_e[:, q, t, h, :], lhsT,
                                wmhi8[:], start=True, stop=False)
                            nc.tensor.matmul(
                                ps_e[:, q, t, h, :], lhsT,
                                wmlo8[:], start=False, stop=True)
            # one contiguous PSUM->SBUF flush (f32 -> bf16) on the vector
            # queue, deferred two chunks so its PE-matmul producers are done
            # by emission time and it never head-of-line blocks anything.
            emb_t = st['emb']
            st['flush'].append(lambda: nc.vector.tensor_copy(
                emb_t[:, q0:q0 + nq], ps_e[:, :nq]))

        def emit_hop(st, hop):
            embt = st['emb']
            u_cur = st['u']
            # u2[p, h, r, s]: s=0 -> u, s=1 -> u*bvec   (bf16)
            u2 = tmp.tile([128, 2, BL, 2], F32, tag="u2")
            nc.vector.tensor_copy(
                out=bass.AP(u2.tensor, u2[:].offset,
                            [u2[:].ap[0], [8, 2], [2, BL]]),
                in_=u_cur[:])
            for h in range(2):
                nc.vector.tensor_scalar_mul(
                    bass.AP(u2.tensor, u2[:].offset + h * 8 + 1,
                            [u2[:].ap[0], [2, BL]]),
                    u_cur[:, h, :], bvec[:, h:h + 1])
            # scores: batch r -> PSUM partition 32r (PE col tile positions)
            ps_sc = pp_s.tile([128, M], F32, tag="scr", bufs=1)
            for r in range(BL):
                k = 0
                for h in range(2):
                    for s in range(2):
                        nc.tensor.matmul(
                            ps_sc[32 * r:32 * r + 1, :],
                            u2[:, h, r, s:s + 1],
                            emb_ap(embt, hop, h, r, s, 'score'),
                            start=(k == 0), stop=(k == 3),
                            tile_position=(0, 32 * r))
                        k += 1
            # masked softmax; rows 32r are live, other lanes carry finite
            # junk (pm row = 0 there) and are never read cross-partition
            sm = tmp.tile([128, M], F32, tag="sm")
            nc.vector.tensor_mul(sm[:], ps_sc[:], pm4[:])
            nc.vector.tensor_add(sm[:], sm[:], pm4_m1[:])
            mx = tmp.tile([128, 1], F32, tag="mx")
            nc.vector.tensor_reduce(mx[:], sm[:], AX.X, AluOpType.max,
                                    negate=True)
            ex = tmp.tile([128, M], F32, tag="ex")
            nc.vector.tensor_scalar_add(ex[:], sm[:], mx[:, 0:1])
            nc.scalar.activation(ex[:], ex[:], AF.Exp)
            nc.vector.tensor_mul(ex[:], ex[:], pm4[:])
            ssum = tmp.tile([128, 1], F32, tag="ssum")
            nc.vector.tensor_reduce(ssum[:], ex[:], AX.X, AluOpType.add)
            nc.vector.tensor_scalar_add(ssum[:], ssum[:], 1e-13)
            nc.vector.reciprocal(ssum[:], ssum[:])
            pw = tmp.tile([128, M], F32, tag="pw")
            nc.vector.tensor_scalar_mul(pw[:], ex[:], ssum[:, 0:1])
            if dump and hop == 0:
                dsc = tmp.tile([128, M], F32, tag=f"dsc", bufs=1)
                nc.vector.tensor_copy(dsc[:], ps_sc[:])
                st['sc0'] = dsc
                dpw = tmp.tile([128, M], F32, tag=f"dpw", bufs=1)
                nc.vector.tensor_copy(dpw[:], pw[:])
                st['pw0'] = dpw
            # broadcast p across partitions: sel_r one-hot picks row 32r
            ps_p = pp_t.tile([128, BL * M], F32, tag="aux")
            for r in range(BL):
                nc.tensor.matmul(ps_p[:, r * M:(r + 1) * M],
                                 sel[:, r, :], pw[:],
                                 start=True, stop=True)
            # o[p,h,s,r] = sum_m embC[p,h,m,r,s] * p[r,m]
            o_t = tmp.tile([128, 2, 2, BL], F32, tag="ot")
            scr = tmp.tile([128, 2, 2, BL, M], F32, tag="scr2", bufs=1)
            psp_v = bass.AP(ps_p.tensor, ps_p[:].offset,
                            [ps_p[:].ap[0], [M, BL], [1, M]])
            for h in range(2):
                for s in range(2):
                    nc.vector.tensor_tensor(
                        scr[:, h, s, :, :],
                        emb_ap(embt, hop + 1, h, None, s, 'ored'),
                        psp_v, AluOpType.mult)
                    nc.vector.tensor_reduce(o_t[:, h, s, :], scr[:, h, s],
                                            AX.X, AluOpType.add)
            # u' = u + o_plain + bvec*o_a
            u_nxt = up.tile([128, 2, BL], F32, tag="u")
            nc.vector.tensor_add(
                u_nxt[:], u_cur[:],
                bass.AP(o_t.tensor, o_t[:].offset,
                        [o_t[:].ap[0], [8, 2], [1, BL]]))
            for h in range(2):
                nc.vector.scalar_tensor_tensor(
                    u_nxt[:, h, :],
                    bass.AP(o_t.tensor, o_t[:].offset + h * 8 + 4,
                            [o_t[:].ap[0], [1, BL]]),
                    bvec[:, h:h + 1],
                    u_nxt[:, h, :],
                    AluOpType.mult, AluOpType.add)
            st['u'] = u_nxt

        def emit_cc_u3(st):
            for h in range(2):
                nc.sync.dma_start(u3_loc[h], st['u'][:, h, :])
            if sim_no_coll:
                nc.sync.dma_start(u3_gth[0], u3_loc)
            else:
                nc.gpsimd.collective_compute(
                    "AllGather", AluOpType.bypass,
                    replica_groups=[list(range(NC_))],
                    ins=[u3_loc], outs=[u3_gth])

        def emit_head_mm(st):
            u3g = tmp.tile([128, 2, B], F32, tag="u3g")
            for h in range(2):
                src = bass.AP(u3_gth.tensor, h * 128 * BL,
                              [[BL, 128], [2 * 128 * BL, NC_], [1, BL]])
                nc.sync.dma_start(u3g[:, h, :], src)
            u3b = tmp.tile([128, 2, B], BF16, tag="u3b")
            nc.vector.tensor_copy(u3b[:], u3g[:])
            ps_wx = pp_w.tile([128, NCH, B], F32, tag="pswx")
            for pb in (32, 64, 96):
                nc.vector.memset(ps_wx[pb:pb + 32, NCH - 1, :], 0.0)
            for ch in range(NCH):
                m_sz = min(CH, VL - ch * CH)
                for h in range(2):
                    nc.tensor.matmul(
                        ps_wx[:m_sz, ch, :],
                        wt_sb[:, h, ch * CH:ch * CH + m_sz],
                        u3b[:, h, :],
                        start=(h == 0), stop=(h == 1))
            st['ps_wx'] = ps_wx

        def emit_bn(st):
            ps_wx = st['ps_wx']
            mean = tmp.tile([128, NCH], F32, tag="mean")
            nc.vector.tensor_reduce(mean[:], ps_wx[:], AX.X, AluOpType.add)
            nc.vector.tensor_scalar_mul(mean[:], mean[:], 1.0 / B)
            sq = tmp.tile([128, NCH, B], F32, tag="sq", bufs=1)
            nc.scalar.activation(sq[:], ps_wx[:], AF.Square)
            var = tmp.tile([128, NCH], F32, tag="var")
            nc.vector.tensor_reduce(var[:], sq[:], AX.X, AluOpType.add)
            nc.vector.tensor_scalar_mul(var[:], var[:], 1.0 / B)
            msq = tmp.tile([128, NCH], F32, tag="msq")
            nc.vector.tensor_mul(msq[:], mean[:], mean[:])
            nc.vector.tensor_sub(var[:], var[:], msq[:])
            rstd = tmp.tile([128, NCH], F32, tag="rstd")
            eps_t = tmp.tile([128, 1], F32, tag="eps")
            nc.vector.memset(eps_t[:], 1e-5)
            nc.scalar.activation(rstd[:], var[:], AF.Sqrt, bias=eps_t[:],
                                 scale=1.0)
            nc.vector.reciprocal(rstd[:], rstd[:])
            av = tmp.tile([128, NCH], F32, tag="av")
            nc.vector.tensor_mul(av[:], gb_sb[:, 0, :], rstd[:])
            bv = tmp.tile([128, NCH], F32, tag="bv")
            nc.vector.tensor_mul(bv[:], av[:], mean[:])
            nc.vector.tensor_sub(bv[:], gb_sb[:, 1, :], bv[:])
            # y = av*wx + (bv + logvm)
            lv2 = tmp.tile([128, NCH, B], F32, tag="lv2", bufs=1)
            bv_b = bass.AP(bv.tensor, bv[:].offset,
                           [bv[:].ap[0], [1, NCH], [0, B]])
            nc.vector.tensor_tensor(lv2[:], logvm[:], bv_b, AluOpType.add)
            y_all = tmp.tile([128, NCH, B], F32, tag="yall", bufs=1)
            av_b = bass.AP(av.tensor, av[:].offset,
                           [av[:].ap[0], [1, NCH], [0, B]])
            nc.vector.tensor_tensor(y_all[:], ps_wx[:], av_b, AluOpType.mult)
            nc.vector.tensor_add(y_all[:], y_all[:], lv2[:])
            st['y_all'] = y_all

        def emit_lse(st):
            y_all = st['y_all']
            es = tmp.tile([128, NCH, B], F32, tag="es", bufs=1)
            nc.scalar.activation(es[:], y_all[:], AF.Exp)
            # pad rows of the last chunk carry y = 0*0 + 0 + log(1e-13), so
            # exp(y) ~ 1e-13 — negligible in the 4000-term sums; no memset.
            xs = tmp.tile([128, B], F32, tag="xs")
            nc.vector.tensor_reduce(
                xs[:], bass.AP(es.tensor, es[:].offset,
                               [es[:].ap[0], [1, B], [B, NCH]]),
                AX.X, AluOpType.add)
            ps_tr = pp_t.tile([B, 128], F32, tag="aux")
            nc.tensor.transpose(ps_tr[:], xs[:], ident[:])
            s_loc = tmp.tile([B, 1], F32, tag="sloc")
            nc.vector.tensor_reduce(s_loc[:], ps_tr[:, :], AX.X, AluOpType.add)
            nc.sync.dma_start(lse_loc, s_loc[:, 0:1])
            st['s_loc'] = s_loc

        def emit_cc_lse(st):
            if sim_no_coll:
                nc.sync.dma_start(
                    bass.AP(lse_gth.tensor, 0, [[1, 1], [1, B]]),
                    st['s_loc'][:, 0:1])
            else:
                nc.gpsimd.collective_compute(
                    "AllGather", AluOpType.bypass,
                    replica_groups=[list(range(NC_))],
                    ins=[lse_loc], outs=[lse_gth])

        def emit_final(st):
            y_all = st['y_all']
            lse8 = tmp.tile([B, NC_], F32, tag="lse8")
            nc.sync.dma_start(lse8[:], bass.AP(lse_gth.tensor, 0,
                                               [[1, B], [B, NC_]]))
            s8 = tmp.tile([B, 1], F32, tag="s8")
            nc.vector.tensor_reduce(s8[:], lse8[:], AX.X, AluOpType.add)
            glse = tmp.tile([B, 1], F32, tag="glse")
            nc.scalar.activation(glse[:], s8[:], AF.Ln)
            ps_g1 = pp_t.tile([1, B], F32, tag="aux")
            nc.tensor.transpose(ps_g1[:], glse[:], ident[:B, :B])
            g_row = tmp.tile([1, B], F32, tag="grow")
            nc.vector.tensor_copy(g_row[:], ps_g1[:])
            ps_gb = pp_t.tile([128, B], F32, tag="aux")
            nc.tensor.matmul(ps_gb[:], ones1[:], g_row[:], start=True,
                             stop=True)
            out_sb = tmp.tile([128, NCH, B], BF16, tag="outsb", bufs=1)
            gb_b = bass.AP(ps_gb.tensor, ps_gb[:].offset,
                           [ps_gb[:].ap[0], [0, NCH], [1, B]])
            nc.vector.tensor_tensor(out_sb[:], y_all[:], gb_b,
                                    AluOpType.subtract)
            nc.sync.dma_start(out, out_sb[:])

        def emit_dump(st):
            d = tmp.tile([128, NCH, B], BF16, tag="outsb", bufs=1)
            nc.vector.memset(d[:], 0.0)
            nc.vector.tensor_copy(
                d[:, 0, 0:8], bass.AP(st['u0'].tensor, st['u0'][:].offset,
                                      [st['u0'][:].ap[0], [1, 8]]))
            nc.vector.tensor_copy(
                d[:, 1, 0:8], bass.AP(st['u'].tensor, st['u'][:].offset,
                                      [st['u'][:].ap[0], [1, 8]]))
            emb_t = st['emb']
            for mm in range(4):
                nc.vector.tensor_copy(
                    d[:, 2 + mm, 0:16],
                    bass.AP(emb_t.tensor, emb_t[:].offset + mm * 64,
                            [emb_t[:].ap[0], [1, 16]]))
            nc.vector.tensor_copy(
                bass.AP(d.tensor, d[:].offset + 8 * B, [d[:].ap[0], [1, M]]),
                st['sc0'][:, 0:M])
            nc.vector.tensor_copy(
                bass.AP(d.tensor, d[:].offset + 12 * B, [d[:].ap[0], [1, M]]),
                st['pw0'][:, 0:M])
            nc.sync.dma_start(out, d[:])

        def tail_segments(st):
            if dump:
                return [
                    lambda: emit_hop(st, 0),
                    lambda: emit_hop(st, 1),
                    lambda: emit_hop(st, 2),
                    lambda: emit_dump(st),
                ] + [lambda: None] * 5
            return [
                lambda: emit_hop(st, 0),
                lambda: emit_hop(st, 1),
                lambda: emit_hop(st, 2),
                lambda: emit_cc_u3(st),
                lambda: emit_head_mm(st),
                lambda: emit_bn(st),
                lambda: emit_lse(st),
                lambda: emit_cc_lse(st),
                lambda: emit_final(st),
            ]

        # segment s emitted after chunk SCHED[s] of the next iteration
        # (-1 = before chunk 0); chosen so each collective's input-ready wait
        # on the gpsimd queue is covered by already-dispatched gather runway
        SCHED = {-1: [0], 0: [1], 2: [2], 6: [3], 7: [4], 8: [5],
                 10: [6], 12: [7, 8]}

        FLUSH_DEPTH = 2
        prev = None
        for rep in range(repeat):
            st = {'flush': []}
            emb_t = embp.tile([128, M, 4, 2, 8], F32, tag="emba",
                              name=f"emba{rep % 2}")
            st['emb'] = emb_t
            segs = tail_segments(prev) if prev is not None else []
            if segs:
                for s in SCHED.get(-1, []):
                    segs[s]()
            for j in range(NCHUNK):
                emit_chunk(st, j)        # appends chunk j's flush
                while len(st['flush']) > FLUSH_DEPTH:
                    st['flush'].pop(0)()
                if segs:
                    for s in SCHED.get(j, []):
                        segs[s]()
            for f in st['flush']:
                f()
            st['flush'] = []
            emit_query(st)
            prev = st
        for seg in tail_segments(prev):
            seg()

    nc.compile()
    return nc


def marshal(inputs):
    """FULL inputs -> per-core in_maps."""
    wmat, wm_hi, wm_lo, bvec, ones1, ident = _consts()
    trainS = np.asarray(inputs['trainS'])
    trainQ = np.asarray(inputs['trainQ'])
    trainVM = np.asarray(inputs['trainVM'], dtype=np.float32)
    trainPM = np.asarray(inputs['trainPM'], dtype=np.float32)
    trainQM = np.asarray(inputs['trainQM'], dtype=np.float32)
    tab1 = np.asarray(inputs['A1'], dtype=np.float32).astype(ml_dtypes.bfloat16)
    tab234 = np.concatenate(
        [np.asarray(inputs[k], dtype=np.float32) for k in ('A2', 'A3', 'A4')],
        axis=1).astype(FP8NP)
    # merged row bytes: [A1 bf16 512B | A234 fp8 768B | pad] as bf16[ROW_W]
    tabm = np.zeros((V, ROW_B), np.uint8)
    tabm[:, :512] = tab1.view(np.uint8).reshape(V, 512)
    tabm[:, 512:1280] = tab234.view(np.uint8).reshape(V, 768)
    tabm = tabm.view(ml_dtypes.bfloat16)
    W = np.asarray(inputs['W'], dtype=np.float32)
    gamma = np.asarray(inputs['gamma'], dtype=np.float32)
    beta = np.asarray(inputs['beta'], dtype=np.float32)

    in_maps = []
    for c in range(NC_):
        rb = slice(BL * c, BL * (c + 1))
        vs = VL * c
        # story indices
        arr = np.zeros((M, BL, 32), np.int16)
        arr[:, :, :L] = trainS[rb].transpose(1, 0, 2)
        sidx = _wrap_idx(arr.reshape(-1))
        # query indices
        qa = np.zeros((BL, 32), np.int16)
        qa[:, :LQ] = trainQ[rb, 0, :]
        qidx = _wrap_idx(qa.reshape(-1))
        # query mask column
        qmc = np.zeros((128, 1), np.float32)
        for r in range(BL):
            qmc[32 * r:32 * r + LQ, 0] = trainQM[BL * c + r]
        # W^T slice: wt[h, p, v] = W[vs+v, 128h+p]
        wt = W[vs:vs + VL].T.reshape(2, 128, VL).astype(ml_dtypes.bfloat16)
        # gamma/beta: gb[p, 0, ch] = gamma[vs + ch*128 + p]
        gb = np.zeros((128, 2, NCH), np.float32)
        gpad = np.zeros(NCH * CH, np.float32); gpad[:VL] = gamma[vs:vs + VL]
        bpad = np.zeros(NCH * CH, np.float32); bpad[:VL] = beta[vs:vs + VL]
        gb[:, 0, :] = gpad.reshape(NCH, CH).T
        gb[:, 1, :] = bpad.reshape(NCH, CH).T
        # VM^T -> log(VM + 1e-13) computed on host (mask transform)
        vmt = np.zeros((128, NCH, B), np.float32)
        lv = np.log(trainVM[:, vs:vs + VL].astype(np.float64)
                    + 1e-13).astype(np.float32).T        # [VL, B]
        lvp = np.zeros((NCH * CH, B), np.float32); lvp[:VL] = lv
        vmt[:] = lvp.reshape(NCH, CH, B).transpose(1, 0, 2)
        pmfa = np.zeros((128, M), np.float32)
        for r in range(BL):
            pmfa[32 * r] = trainPM[BL * c + r]
        in_maps.append({
            'tabm': tabm,
            'sidx': sidx, 'qidx': qidx, 'qmc': qmc,
            'pmf': pmfa,
            'wti': wt, 'gbi': gb, 'vmi': vmt,
            'wmi': wmat.astype(ml_dtypes.bfloat16),
            'wmh8i': wm_hi.astype(FP8NP), 'wml8i': wm_lo.astype(FP8NP),
            'bvi': bvec,
            'on1': ones1, 'idi': ident,
        })
    return in_maps


def unmarshal(results):
    outf = np.zeros((B, V), np.float32)
    for c in range(NC_):
        o = np.asarray(results[c]['out']).astype(np.float32)
        o = o.reshape(128, NCH, B)
        outf[:, VL * c:VL * (c + 1)] = \
            o.transpose(2, 1, 0).reshape(B, NCH * CH)[:, :VL]
    return outf


def kernel(**inputs):
    if 'nc' not in _cache:
        _cache['nc'] = build_nc()
    nc = _cache['nc']
    in_maps = marshal(inputs)
    res = run_bass_kernel_spmd(nc, in_maps, list(range(NC_)))
    return unmarshal(res.results)



# revision 42
# speedup vs baseline: 2.1030x; 2.1030x over previous
"""MemN2N (3-hop memory network) forward pass on 8 Trainium2 NeuronCores.

Strategy (per spec sharding hint):
 - Data-parallel over batch (4 batches/core) for the embedding gathers and
   hop math. Embedding tables are replicated in each core's DRAM, merged
   into one [V, 1536B] row = [A1 bf16 | A2,A3,A4 fp8-e4m3 | pad]: one
   gather fetches all 4 tables per word. A1 must stay bf16 (the hop-1
   score path is precision-sensitive: fp8 A1 gives 3e-1 max-rel error vs
   9.6e-3 with this split; gate is 2e-2). Rows are padded 1280->1536B so
   every gather element is 512B-aligned (~25% faster gathers measured).
   The A2..A4 word-contractions run as fp8 x fp8 matmuls (bf16-mixed
   operands hit a 5-6x slower PE path on HW); the position-encoding
   weight column a_l = l-15.5 is split hi=trunc(a), lo=a-hi, both
   e4m3-exact, accumulated over two matmul passes in PSUM.
 - Position encoding pe[l,e] = 1 + a_l*b_e is rank-2, so each embedding
   reduction over words L needs only two weighted sums (plain and a-weighted),
   computed on the TensorEngine with a block-diagonal weight matrix
   (K = 4 batches x 32 word-slots). Results land in PSUM as
   [e-half partitions, (q, t, h, r*2+s)] and are flushed to SBUF in one
   contiguous copy per chunk on the vector queue.
 - After the hops, u3 (4,256) is AllGathered; each core then computes the
   final linear for its 4000-vocab shard, BatchNorm over the full batch,
   and a log-softmax whose log-sum-exp is combined across cores via a second
   tiny AllGather of per-core per-batch sum-exp values.
 - The repeat loop is software-pipelined: iteration k's gathers/emb phase is
   emitted interleaved with iteration k-1's hops/head/collectives so the DMA
   engines (the ~74us/iter roofline) never idle during the serial tail.

kernel(**inputs) takes the FULL unsharded inputs and returns the FULL
(32, 32000) float32 output.
"""
import sys
sys.path.insert(0, '/opt/trn_rl_repo')
import numpy as np
import ml_dtypes
from contextlib import ExitStack

import concourse.bass as bass
import concourse.bacc as bacc
import concourse.tile as tile
from concourse import mybir
from concourse.alu_op_type import AluOpType
from concourse.bass_utils import run_bass_kernel_spmd

F32 = mybir.dt.float32
BF16 = mybir.dt.bfloat16
F8 = mybir.dt.float8e4
I16 = mybir.dt.int16
AF = mybir.ActivationFunctionType
AX = mybir.AxisListType

# fp8 e4m3 numpy dtype matching mybir.dt.float8e4 on device
FP8NP = ml_dtypes.float8_e4m3

# problem constants
B, M, L, LQ, E, V = 32, 100, 30, 30, 256, 32000
NC_ = 8
BL = B // NC_          # 4 local batches
VL = V // NC_          # 4000 local vocab
CH = 128               # head vocab-chunk size (31 full + 1 of 32 = 4000)
NCH = 32               # chunks (last chunk only 32 rows)
NQ = 8                 # gather chunk: 8 m-columns -> 1024 indices
EH = E // 2            # 128, e-half
NCHUNK = (M + NQ - 1) // NQ   # 13

_cache = {}

# PE contraction mode for the fp8 tables (A2..A4):
#   'mixed' : lhsT fp8, rhs = bf16 wmat (single pass)
#   'fp8_2p': lhsT fp8, rhs = fp8 wmat split hi/lo (2 accumulating passes;
#             exact because a_l = int + 0.5 and both parts are e4m3-exact)
MM_MODE = 'fp8_2p'
# merged table row: [A1 bf16 512B | A2|A3|A4 fp8 768B | pad to ROW_B]
ROW_B = 1536
ROW_W = ROW_B // 2     # bf16 units


def _consts():
    a = (np.arange(1, L + 1, dtype=np.float64) - (L + 1) / 2.0)        # (30,)
    b = 4.0 * (np.arange(1, E + 1, dtype=np.float64) - (E + 1) / 2.0) / (E * L)
    wmat = np.zeros((128, 8), np.float32)
    for r in range(4):
        for l in range(L):
            wmat[32 * r + l, 2 * r + 0] = 1.0
            wmat[32 * r + l, 2 * r + 1] = a[l]
    # e4m3-exact hi/lo split of wmat: hi = [1-col, trunc(a)], lo = [0, a-hi]
    wm_hi = np.zeros((128, 8), np.float32)
    wm_lo = np.zeros((128, 8), np.float32)
    for r in range(4):
        for l in range(L):
            wm_hi[32 * r + l, 2 * r + 0] = 1.0
            wm_hi[32 * r + l, 2 * r + 1] = np.trunc(a[l])
            wm_lo[32 * r + l, 2 * r + 1] = a[l] - np.trunc(a[l])
    bvec = b.astype(np.float32).reshape(2, EH).T.copy()               # [128,2]
    ones1 = np.ones((1, 128), np.float32)
    ident = np.eye(128, dtype=np.float32)
    return wmat, wm_hi, wm_lo, bvec, ones1, ident


def _wrap_idx(flat):
    """int16 flat index array (len % 16 == 0) -> [128, len//16] wrapped+tiled."""
    n = flat.shape[0]
    wr = flat.reshape(n // 16, 16).T.astype(np.int16)   # [16, n//16]
    return np.tile(wr, (8, 1)).copy()                   # [128, n//16]


def build_nc(repeat=1, sim_no_coll=False, dump=False, mm_mode=None):
    mm_mode = mm_mode or MM_MODE
    nc = bacc.Bacc("TRN2", target_bir_lowering=False, debug=False,
                   num_devices=NC_, dynamic_dma_scratch_size=65536)

    # ---- DRAM I/O ----
    # Merged table row = [A1 bf16 512B | A2 fp8 256B | A3 fp8 256B | A4 fp8
    # 256B | pad] = ROW_B (512-aligned rows measure ~30% faster to gather
    # than packed-1280B). A1 stays bf16 (hop-1 score side is precision-
    # sensitive); A2..A4 as fp8 e4m3 (measured end-to-end 9.6e-3 < 2e-2).
    # One gather per chunk keeps SWDGE descriptor-gen at the v1 rate.
    tabm = nc.dram_tensor("tabm", [V, ROW_W], BF16, kind="ExternalInput").ap()
    sidx = nc.dram_tensor("sidx", [128, M * 128 // 16], I16, kind="ExternalInput").ap()
    qidx = nc.dram_tensor("qidx", [128, 8], I16, kind="ExternalInput").ap()
    qmc = nc.dram_tensor("qmc", [128, 1], F32, kind="ExternalInput").ap()
    pmf = nc.dram_tensor("pmf", [128, M], F32, kind="ExternalInput").ap()
    wti = nc.dram_tensor("wti", [2, 128, VL], BF16, kind="ExternalInput").ap()
    gbi = nc.dram_tensor("gbi", [128, 2, NCH], F32, kind="ExternalInput").ap()
    vmi = nc.dram_tensor("vmi", [128, NCH, B], F32, kind="ExternalInput").ap()
    wmi = nc.dram_tensor("wmi", [128, 8], BF16, kind="ExternalInput").ap()
    wmh8i = nc.dram_tensor("wmh8i", [128, 8], F8, kind="ExternalInput").ap()
    wml8i = nc.dram_tensor("wml8i", [128, 8], F8, kind="ExternalInput").ap()
    bvi = nc.dram_tensor("bvi", [128, 2], F32, kind="ExternalInput").ap()
    on1 = nc.dram_tensor("on1", [1, 128], F32, kind="ExternalInput").ap()
    idi = nc.dram_tensor("idi", [128, 128], F32, kind="ExternalInput").ap()
    out = nc.dram_tensor("out", [128, NCH, B], BF16,
                         kind="ExternalOutput").ap()

    u3_loc = nc.dram_tensor("u3_loc", [2, 128, BL], F32).ap()
    u3_gth = nc.dram_tensor("u3_gth", [NC_, 2, 128, BL], F32,
                            addr_space="Shared").ap()
    lse_loc = nc.dram_tensor("lse_loc", [B], F32).ap()
    lse_gth = nc.dram_tensor("lse_gth", [NC_, B], F32,
                             addr_space="Shared").ap()

    with tile.TileContext(nc) as tc, ExitStack() as ctx:
        cons = ctx.enter_context(tc.tile_pool(name="cons", bufs=1))
        embp = ctx.enter_context(tc.tile_pool(name="embp", bufs=2))
        rt_p = ctx.enter_context(tc.tile_pool(name="rt", bufs=3))
        tmp = ctx.enter_context(tc.tile_pool(name="tmp", bufs=2))
        up = ctx.enter_context(tc.tile_pool(name="up", bufs=2))
        pp_e = ctx.enter_context(tc.tile_pool(name="pp_e", bufs=4, space="PSUM"))
        pp_s = ctx.enter_context(tc.tile_pool(name="pp_s", bufs=1, space="PSUM"))
        pp_w = ctx.enter_context(tc.tile_pool(name="pp_w", bufs=1, space="PSUM"))
        pp_t = ctx.enter_context(tc.tile_pool(name="pp_t", bufs=1, space="PSUM"))

        # ---- constants / small inputs (loaded once) ----
        sidx_sb = cons.tile([128, M * 8], I16)
        nc.sync.dma_start(sidx_sb[:], sidx)
        qidx_sb = cons.tile([128, 8], I16)
        nc.sync.dma_start(qidx_sb[:], qidx)
        wmat = cons.tile([128, 8], BF16)
        nc.sync.dma_start(wmat[:], wmi)
        wmhi8 = cons.tile([128, 8], F8)
        nc.sync.dma_start(wmhi8[:], wmh8i)
        wmlo8 = cons.tile([128, 8], F8)
        nc.sync.dma_start(wmlo8[:], wml8i)
        bvec = cons.tile([128, 2], F32)
        nc.sync.dma_start(bvec[:], bvi)
        ones1 = cons.tile([1, 128], F32)
        nc.sync.dma_start(ones1[:], on1)
        qm_sb = cons.tile([128, 1], F32)
        nc.sync.dma_start(qm_sb[:], qmc)
        ident = cons.tile([128, 128], F32)
        nc.scalar.dma_start(ident[:], idi)
        gb_sb = cons.tile([128, 2, NCH], F32)
        nc.scalar.dma_start(gb_sb[:], gbi)
        wt_sb = cons.tile([128, 2, VL], BF16)
        nc.scalar.dma_start(wt_sb[:, 0, :], wti[0])
        nc.scalar.dma_start(wt_sb[:, 1, :], wti[1])
        logvm = cons.tile([128, NCH, B], F32)
        nc.scalar.dma_start(logvm[:], vmi)

        # loop-invariant derived tiles
        pm4 = cons.tile([128, M], F32)
        nc.sync.dma_start(pm4[:], pmf)
        pm4_m1 = cons.tile([128, M], F32)
        nc.vector.tensor_scalar(pm4_m1[:], pm4[:], -1.0, 1e30,
                                AluOpType.add, AluOpType.mult)
        wmatq = cons.tile([128, 8], BF16)
        nc.vector.tensor_scalar_mul(wmatq[:], wmat[:], qm_sb[:, 0:1])
        # one-hot selector matrices: sel[p, r, q] = (p == 32r), used to
        # replicate softmax row 32r across all 128 partitions via matmul
        sel = cons.tile([128, BL, 128], F32)
        nc.vector.memset(sel[:], 0.0)
        for r in range(BL):
            nc.vector.memset(sel[32 * r:32 * r + 1, r, :], 1.0)
        # scrub the score PSUM bank once: all later writes are finite, so
        # masked-lane arithmetic (0 * stale) can never see boot inf/NaN
        ps_z = pp_s.tile([128, M], F32, tag="scr", bufs=1)
        nc.vector.memset(ps_z[:], 0.0)

        def emb_ap(embt, t, h, r, s, which):
            """AP views of emb_all [128, M, 4, 2, 8] f32; rs index = r*2+s."""
            off = embt[:].offset + t * 16 + h * 8 + (0 if r is None else r * 2) + s
            if which == 'score':       # [128, M] for fixed (t,h,r,s)
                return bass.AP(embt.tensor, off, [embt[:].ap[0], [64, M]])
            if which == 'ored':        # [128, BL, M] for fixed (t,h,s)
                return bass.AP(embt.tensor, off,
                               [embt[:].ap[0], [2, BL], [64, M]])
            raise ValueError(which)

        def emit_query(st):
            """Query encoding -> st['u'] (f32 [128, 2, BL])."""
            rq = rt_p.tile([128, 1, E], BF16, tag="rq")
            tabm_q = bass.AP(tabm.tensor, 0, [[ROW_W, V], [1, E]])
            nc.gpsimd.dma_gather(rq[:], tabm_q, qidx_sb[:, :],
                                 num_idxs=128, num_idxs_reg=128,
                                 elem_size=E, elem_step=ROW_W)
            ps_q = pp_s.tile([128, 16], F32, tag="scr", bufs=1)
            for h in range(2):
                nc.tensor.matmul(ps_q[:, h * 8:(h + 1) * 8],
                                 rq[:, 0, h * EH:(h + 1) * EH],
                                 wmatq[:], start=True, stop=True)
            q_sb = tmp.tile([128, 16], F32, tag="q_sb")
            nc.vector.tensor_copy(q_sb[:], ps_q[:])
            u_cur = up.tile([128, 2, BL], F32, tag="u")
            for h in range(2):
                psq_odd = bass.AP(q_sb.tensor, q_sb[:].offset + h * 8 + 1,
                                  [q_sb[:].ap[0], [2, BL]])
                psq_evn = bass.AP(q_sb.tensor, q_sb[:].offset + h * 8 + 0,
                                  [q_sb[:].ap[0], [2, BL]])
                nc.vector.scalar_tensor_tensor(
                    u_cur[:, h, :], psq_odd, bvec[:, h:h + 1], psq_evn,
                    AluOpType.mult, AluOpType.add)
            st['u'] = u_cur
            st['u0'] = u_cur

        def emit_chunk(st, j):
            """Gather chunk j + PE-reduce; flush is deferred (st['flush'])."""
            q0 = j * NQ
            nq = min(NQ, M - q0)
            # read only the 1280B payload of each 1536B-aligned row — the
            # row START stays 512-aligned; the trailing 256B pad is skipped
            rt = rt_p.tile([128, NQ, 640], BF16, tag="rt")
            tab_pl = bass.AP(tabm.tensor, 0, [[ROW_W, V], [1, 640]])
            nc.gpsimd.dma_gather(
                rt[:, :nq, :], tab_pl,
                sidx_sb[:, q0 * 8:(q0 + nq) * 8],
                num_idxs=nq * 128, num_idxs_reg=nq * 128,
                elem_size=640, elem_step=ROW_W)
            ps_e = pp_e.tile([128, NQ, 4, 2, 8], F32, tag="pse")
            for q in range(nq):
                for t in range(4):
                    for h in range(2):
                        if t == 0:
                            nc.tensor.matmul(
                                ps_e[:, q, t, h, :],
                                rt[:, q, h * EH:(h + 1) * EH],
                                wmat[:], start=True, stop=True)
                            continue
                        # fp8 bytes live at bf16-element offset
                        # 256 + (t-1)*128 + h*64; 64 bf16 = 128 fp8
                        b0 = 256 + (t - 1) * 128 + h * 64
                        lhsT = rt[:, q, b0:b0 + 64].bitcast(F8)
                        if mm_mode == 'mixed':
                            nc.tensor.matmul(
                                ps_e[:, q, t, h, :], lhsT,
                                wmat[:], start=True, stop=True)
                        else:
                            nc.tensor.matmul(
                                ps_e[:, q, t, h, :], lhsT,
                                wmhi8[:], start=True, stop=False)
                            nc.tensor.matmul(
                                ps_e[:, q, t, h, :], lhsT,
                                wmlo8[:], start=False, stop=True)
            # one contiguous PSUM->SBUF flush (f32 -> bf16) on the vector
            # queue, deferred two chunks so its PE-matmul producers are done
            # by emission time and it never head-of-line blocks anything.
            emb_t = st['emb']
            st['flush'].append(lambda: nc.vector.tensor_copy(
                emb_t[:, q0:q0 + nq], ps_e[:, :nq]))

        def emit_hop(st, hop):
            embt = st['emb']
            u_cur = st['u']
            # u2[p, h, r, s]: s=0 -> u, s=1 -> u*bvec   (bf16)
            u2 = tmp.tile([128, 2, BL, 2], F32, tag="u2")
            nc.vector.tensor_copy(
                out=bass.AP(u2.tensor, u2[:].offset,
                            [u2[:].ap[0], [8, 2], [2, BL]]),
                in_=u_cur[:])
            for h in range(2):
                nc.vector.tensor_scalar_mul(
                    bass.AP(u2.tensor, u2[:].offset + h * 8 + 1,
                            [u2[:].ap[0], [2, BL]]),
                    u_cur[:, h, :], bvec[:, h:h + 1])
            # scores: batch r -> PSUM partition 32r (PE col tile positions)
            ps_sc = pp_s.tile([128, M], F32, tag="scr", bufs=1)
            for r in range(BL):
                k = 0
                for h in range(2):
                    for s in range(2):
                        nc.tensor.matmul(
                            ps_sc[32 * r:32 * r + 1, :],
                            u2[:, h, r, s:s + 1],
                            emb_ap(embt, hop, h, r, s, 'score'),
                            start=(k == 0), stop=(k == 3),
                            tile_position=(0, 32 * r))
                        k += 1
            # masked softmax; rows 32r are live, other lanes carry finite
            # junk (pm row = 0 there) and are never read cross-partition
            sm = tmp.tile([128, M], F32, tag="sm")
            nc.vector.tensor_mul(sm[:], ps_sc[:], pm4[:])
            nc.vector.tensor_add(sm[:], sm[:], pm4_m1[:])
            mx = tmp.tile([128, 1], F32, tag="mx")
            nc.vector.tensor_reduce(mx[:], sm[:], AX.X, AluOpType.max,
                                    negate=True)
            ex = tmp.tile([128, M], F32, tag="ex")
            nc.vector.tensor_scalar_add(ex[:], sm[:], mx[:, 0:1])
            nc.scalar.activation(ex[:], ex[:], AF.Exp)
            nc.vector.tensor_mul(ex[:], ex[:], pm4[:])
            ssum = tmp.tile([128, 1], F32, tag="ssum")
            nc.vector.tensor_reduce(ssum[:], ex[:], AX.X, AluOpType.add)
            nc.vector.tensor_scalar_add(ssum[:], ssum[:], 1e-13)
            nc.vector.reciprocal(ssum[:], ssum[:])
            pw = tmp.tile([128, M], F32, tag="pw")
            nc.vector.tensor_scalar_mul(pw[:], ex[:], ssum[:, 0:1])
            if dump and hop == 0:
                dsc = tmp.tile([128, M], F32, tag=f"dsc", bufs=1)
                nc.vector.tensor_copy(dsc[:], ps_sc[:])
                st['sc0'] = dsc
                dpw = tmp.tile([128, M], F32, tag=f"dpw", bufs=1)
                nc.vector.tensor_copy(dpw[:], pw[:])
                st['pw0'] = dpw
            # broadcast p across partitions: sel_r one-hot picks row 32r
            ps_p = pp_t.tile([128, BL * M], F32, tag="aux")
            for r in range(BL):
                nc.tensor.matmul(ps_p[:, r * M:(r + 1) * M],
                                 sel[:, r, :], pw[:],
                                 start=True, stop=True)
            # o[p,h,s,r] = sum_m embC[p,h,m,r,s] * p[r,m]
            o_t = tmp.tile([128, 2, 2, BL], F32, tag="ot")
            scr = tmp.tile([128, 2, 2, BL, M], F32, tag="scr2", bufs=1)
            psp_v = bass.AP(ps_p.tensor, ps_p[:].offset,
                            [ps_p[:].ap[0], [M, BL], [1, M]])
            for h in range(2):
                for s in range(2):
                    nc.vector.tensor_tensor(
                        scr[:, h, s, :, :],
                        emb_ap(embt, hop + 1, h, None, s, 'ored'),
                        psp_v, AluOpType.mult)
                    nc.vector.tensor_reduce(o_t[:, h, s, :], scr[:, h, s],
                                            AX.X, AluOpType.add)
            # u' = u + o_plain + bvec*o_a
            u_nxt = up.tile([128, 2, BL], F32, tag="u")
            nc.vector.tensor_add(
                u_nxt[:], u_cur[:],
                bass.AP(o_t.tensor, o_t[:].offset,
                        [o_t[:].ap[0], [8, 2], [1, BL]]))
            for h in range(2):
                nc.vector.scalar_tensor_tensor(
                    u_nxt[:, h, :],
                    bass.AP(o_t.tensor, o_t[:].offset + h * 8 + 4,
                            [o_t[:].ap[0], [1, BL]]),
                    bvec[:, h:h + 1],
                    u_nxt[:, h, :],
                    AluOpType.mult, AluOpType.add)
            st['u'] = u_nxt

        def emit_cc_u3(st):
            for h in range(2):
                nc.sync.dma_start(u3_loc[h], st['u'][:, h, :])
            if sim_no_coll:
                nc.sync.dma_start(u3_gth[0], u3_loc)
            else:
                nc.gpsimd.collective_compute(
                    "AllGather", AluOpType.bypass,
                    replica_groups=[list(range(NC_))],
                    ins=[u3_loc], outs=[u3_gth])

        def emit_head_mm(st):
            u3g = tmp.tile([128, 2, B], F32, tag="u3g")
            for h in range(2):
                src = bass.AP(u3_gth.tensor, h * 128 * BL,
                              [[BL, 128], [2 * 128 * BL, NC_], [1, BL]])
                nc.sync.dma_start(u3g[:, h, :], src)
            u3b = tmp.tile([128, 2, B], BF16, tag="u3b")
            nc.vector.tensor_copy(u3b[:], u3g[:])
            ps_wx = pp_w.tile([128, NCH, B], F32, tag="pswx")
            for pb in (32, 64, 96):
                nc.vector.memset(ps_wx[pb:pb + 32, NCH - 1, :], 0.0)
            for ch in range(NCH):
                m_sz = min(CH, VL - ch * CH)
                for h in range(2):
                    nc.tensor.matmul(
                        ps_wx[:m_sz, ch, :],
                        wt_sb[:, h, ch * CH:ch * CH + m_sz],
                        u3b[:, h, :],
                        start=(h == 0), stop=(h == 1))
            st['ps_wx'] = ps_wx

        def emit_bn(st):
            ps_wx = st['ps_wx']
            mean = tmp.tile([128, NCH], F32, tag="mean")
            nc.vector.tensor_reduce(mean[:], ps_wx[:], AX.X, AluOpType.add)
            nc.vector.tensor_scalar_mul(mean[:], mean[:], 1.0 / B)
            sq = tmp.tile([128, NCH, B], F32, tag="sq", bufs=1)
            nc.scalar.activation(sq[:], ps_wx[:], AF.Square)
            var = tmp.tile([128, NCH], F32, tag="var")
            nc.vector.tensor_reduce(var[:], sq[:], AX.X, AluOpType.add)
            nc.vector.tensor_scalar_mul(var[:], var[:], 1.0 / B)
            msq = tmp.tile([128, NCH], F32, tag="msq")
            nc.vector.tensor_mul(msq[:], mean[:], mean[:])
            nc.vector.tensor_sub(var[:], var[:], msq[:])
            rstd = tmp.tile([128, NCH], F32, tag="rstd")
            eps_t = tmp.tile([128, 1], F32, tag="eps")
            nc.vector.memset(eps_t[:], 1e-5)
            nc.scalar.activation(rstd[:], var[:], AF.Sqrt, bias=eps_t[:],
                                 scale=1.0)
            nc.vector.reciprocal(rstd[:], rstd[:])
            av = tmp.tile([128, NCH], F32, tag="av")
            nc.vector.tensor_mul(av[:], gb_sb[:, 0, :], rstd[:])
            bv = tmp.tile([128, NCH], F32, tag="bv")
            nc.vector.tensor_mul(bv[:], av[:], mean[:])
            nc.vector.tensor_sub(bv[:], gb_sb[:, 1, :], bv[:])
            # y = av*wx + (bv + logvm)
            lv2 = tmp.tile([128, NCH, B], F32, tag="lv2", bufs=1)
            bv_b = bass.AP(bv.tensor, bv[:].offset,
                           [bv[:].ap[0], [1, NCH], [0, B]])
            nc.vector.tensor_tensor(lv2[:], logvm[:], bv_b, AluOpType.add)
            y_all = tmp.tile([128, NCH, B], F32, tag="yall", bufs=1)
            av_b = bass.AP(av.tensor, av[:].offset,
                           [av[:].ap[0], [1, NCH], [0, B]])
            nc.vector.tensor_tensor(y_all[:], ps_wx[:], av_b, AluOpType.mult)
            nc.vector.tensor_add(y_all[:], y_all[:], lv2[:])
            st['y_all'] = y_all

        def emit_lse(st):
            y_all = st['y_all']
            es = tmp.tile([128, NCH, B], F32, tag="es", bufs=1)
            nc.scalar.activation(es[:], y_all[:], AF.Exp)
            # pad rows of the last chunk carry y = 0*0 + 0 + log(1e-13), so
            # exp(y) ~ 1e-13 — negligible in the 4000-term sums; no memset.
            xs = tmp.tile([128, B], F32, tag="xs")
            nc.vector.tensor_reduce(
                xs[:], bass.AP(es.tensor, es[:].offset,
                               [es[:].ap[0], [1, B], [B, NCH]]),
                AX.X, AluOpType.add)
            ps_tr = pp_t.tile([B, 128], F32, tag="aux")
            nc.tensor.transpose(ps_tr[:], xs[:], ident[:])
            s_loc = tmp.tile([B, 1], F32, tag="sloc")
            nc.vector.tensor_reduce(s_loc[:], ps_tr[:, :], AX.X, AluOpType.add)
            nc.sync.dma_start(lse_loc, s_loc[:, 0:1])
            st['s_loc'] = s_loc

        def emit_cc_lse(st):
            if sim_no_coll:
                nc.sync.dma_start(
                    bass.AP(lse_gth.tensor, 0, [[1, 1], [1, B]]),
                    st['s_loc'][:, 0:1])
            else:
                nc.gpsimd.collective_compute(
                    "AllGather", AluOpType.bypass,
                    replica_groups=[list(range(NC_))],
                    ins=[lse_loc], outs=[lse_gth])

        def emit_final(st):
            y_all = st['y_all']
            lse8 = tmp.tile([B, NC_], F32, tag="lse8")
            nc.sync.dma_start(lse8[:], bass.AP(lse_gth.tensor, 0,
                                               [[1, B], [B, NC_]]))
            s8 = tmp.tile([B, 1], F32, tag="s8")
            nc.vector.tensor_reduce(s8[:], lse8[:], AX.X, AluOpType.add)
            glse = tmp.tile([B, 1], F32, tag="glse")
            nc.scalar.activation(glse[:], s8[:], AF.Ln)
            ps_g1 = pp_t.tile([1, B], F32, tag="aux")
            nc.tensor.transpose(ps_g1[:], glse[:], ident[:B, :B])
            g_row = tmp.tile([1, B], F32, tag="grow")
            nc.vector.tensor_copy(g_row[:], ps_g1[:])
            ps_gb = pp_t.tile([128, B], F32, tag="aux")
            nc.tensor.matmul(ps_gb[:], ones1[:], g_row[:], start=True,
                             stop=True)
            out_sb = tmp.tile([128, NCH, B], BF16, tag="outsb", bufs=1)
            gb_b = bass.AP(ps_gb.tensor, ps_gb[:].offset,
                           [ps_gb[:].ap[0], [0, NCH], [1, B]])
            nc.vector.tensor_tensor(out_sb[:], y_all[:], gb_b,
                                    AluOpType.subtract)
            nc.sync.dma_start(out, out_sb[:])

        def emit_dump(st):
            d = tmp.tile([128, NCH, B], BF16, tag="outsb", bufs=1)
            nc.vector.memset(d[:], 0.0)
            nc.vector.tensor_copy(
                d[:, 0, 0:8], bass.AP(st['u0'].tensor, st['u0'][:].offset,
                                      [st['u0'][:].ap[0], [1, 8]]))
            nc.vector.tensor_copy(
                d[:, 1, 0:8], bass.AP(st['u'].tensor, st['u'][:].offset,
                                      [st['u'][:].ap[0], [1, 8]]))
            emb_t = st['emb']
            for mm in range(4):
                nc.vector.tensor_copy(
                    d[:, 2 + mm, 0:16],
                    bass.AP(emb_t.tensor, emb_t[:].offset + mm * 64,
                            [emb_t[:].ap[0], [1, 16]]))
            nc.vector.tensor_copy(
                bass.AP(d.tensor, d[:].offset + 8 * B, [d[:].ap[0], [1, M]]),
                st['sc0'][:, 0:M])
            nc.vector.tensor_copy(
                bass.AP(d.tensor, d[:].offset + 12 * B, [d[:].ap[0], [1, M]]),
                st['pw0'][:, 0:M])
            nc.sync.dma_start(out, d[:])

        def tail_segments(st):
            if dump:
                return [
                    lambda: emit_hop(st, 0),
                    lambda: emit_hop(st, 1),
                    lambda: emit_hop(st, 2),
                    lambda: emit_dump(st),
                ] + [lambda: None] * 5
            return [
                lambda: emit_hop(st, 0),
                lambda: emit_hop(st, 1),
                lambda: emit_hop(st, 2),
                lambda: emit_cc_u3(st),
                lambda: emit_head_mm(st),
                lambda: emit_bn(st),
                lambda: emit_lse(st),
                lambda: emit_cc_lse(st),
                lambda: emit_final(st),
            ]

        # segment s emitted after chunk SCHED[s] of the next iteration
        # (-1 = before chunk 0); chosen so each collective's input-ready wait
        # on the gpsimd queue is covered by already-dispatched gather runway
        SCHED = {-1: [0], 0: [1], 2: [2], 6: [3], 7: [4], 8: [5],
                 10: [6], 12: [7, 8]}

        FLUSH_DEPTH = 2
        prev = None
        for rep in range(repeat):
            st = {'flush': []}
            emb_t = embp.tile([128, M, 4, 2, 8], F32, tag="emba",
                              name=f"emba{rep % 2}")
            st['emb'] = emb_t
            segs = tail_segments(prev) if prev is not None else []
            if segs:
                for s in SCHED.get(-1, []):
                    segs[s]()
            for j in range(NCHUNK):
                emit_chunk(st, j)        # appends chunk j's flush
                while len(st['flush']) > FLUSH_DEPTH:
                    st['flush'].pop(0)()
                if segs:
                    for s in SCHED.get(j, []):
                        segs[s]()
            for f in st['flush']:
                f()
            st['flush'] = []
            emit_query(st)
            prev = st
        for seg in tail_segments(prev):
            seg()

    nc.compile()
    return nc


def marshal(inputs):
    """FULL inputs -> per-core in_maps."""
    wmat, wm_hi, wm_lo, bvec, ones1, ident = _consts()
    trainS = np.asarray(inputs['trainS'])
    trainQ = np.asarray(inputs['trainQ'])
    trainVM = np.asarray(inputs['trainVM'], dtype=np.float32)
    trainPM = np.asarray(inputs['trainPM'], dtype=np.float32)
    trainQM = np.asarray(inputs['trainQM'], dtype=np.float32)
    tab1 = np.asarray(inputs['A1'], dtype=np.float32).astype(ml_dtypes.bfloat16)
    tab234 = np.concatenate(
        [np.asarray(inputs[k], dtype=np.float32) for k in ('A2', 'A3', 'A4')],
        axis=1).astype(FP8NP)
    # merged row bytes: [A1 bf16 512B | A234 fp8 768B | pad] as bf16[ROW_W]
    tabm = np.zeros((V, ROW_B), np.uint8)
    tabm[:, :512] = tab1.view(np.uint8).reshape(V, 512)
    tabm[:, 512:1280] = tab234.view(np.uint8).reshape(V, 768)
    tabm = tabm.view(ml_dtypes.bfloat16)
    W = np.asarray(inputs['W'], dtype=np.float32)
    gamma = np.asarray(inputs['gamma'], dtype=np.float32)
    beta = np.asarray(inputs['beta'], dtype=np.float32)

    in_maps = []
    for c in range(NC_):
        rb = slice(BL * c, BL * (c + 1))
        vs = VL * c
        # story indices
        arr = np.zeros((M, BL, 32), np.int16)
        arr[:, :, :L] = trainS[rb].transpose(1, 0, 2)
        sidx = _wrap_idx(arr.reshape(-1))
        # query indices
        qa = np.zeros((BL, 32), np.int16)
        qa[:, :LQ] = trainQ[rb, 0, :]
        qidx = _wrap_idx(qa.reshape(-1))
        # query mask column
        qmc = np.zeros((128, 1), np.float32)
        for r in range(BL):
            qmc[32 * r:32 * r + LQ, 0] = trainQM[BL * c + r]
        # W^T slice: wt[h, p, v] = W[vs+v, 128h+p]
        wt = W[vs:vs + VL].T.reshape(2, 128, VL).astype(ml_dtypes.bfloat16)
        # gamma/beta: gb[p, 0, ch] = gamma[vs + ch*128 + p]
        gb = np.zeros((128, 2, NCH), np.float32)
        gpad = np.zeros(NCH * CH, np.float32); gpad[:VL] = gamma[vs:vs + VL]
        bpad = np.zeros(NCH * CH, np.float32); bpad[:VL] = beta[vs:vs + VL]
        gb[:, 0, :] = gpad.reshape(NCH, CH).T
        gb[:, 1, :] = bpad.reshape(NCH, CH).T
        # VM^T -> log(VM + 1e-13) computed on host (mask transform)
        vmt = np.zeros((128, NCH, B), np.float32)
        lv = np.log(trainVM[:, vs:vs + VL].astype(np.float64)
                    + 1e-13).astype(np.float32).T        # [VL, B]
        lvp = np.zeros((NCH * CH, B), np.float32); lvp[:VL] = lv
        vmt[:] = lvp.reshape(NCH, CH, B).transpose(1, 0, 2)
        pmfa = np.zeros((128, M), np.float32)
        for r in range(BL):
            pmfa[32 * r] = trainPM[BL * c + r]
        in_maps.append({
            'tabm': tabm,
            'sidx': sidx, 'qidx': qidx, 'qmc': qmc,
            'pmf': pmfa,
            'wti': wt, 'gbi': gb, 'vmi': vmt,
            'wmi': wmat.astype(ml_dtypes.bfloat16),
            'wmh8i': wm_hi.astype(FP8NP), 'wml8i': wm_lo.astype(FP8NP),
            'bvi': bvec,
            'on1': ones1, 'idi': ident,
        })
    return in_maps


def unmarshal(results):
    outf = np.zeros((B, V), np.float32)
    for c in range(NC_):
        o = np.asarray(results[c]['out']).astype(np.float32)
        o = o.reshape(128, NCH, B)
        outf[:, VL * c:VL * (c + 1)] = \
            o.transpose(2, 1, 0).reshape(B, NCH * CH)[:, :VL]
    return outf


def kernel(**inputs):
    if 'nc' not in _cache:
        _cache['nc'] = build_nc()
    nc = _cache['nc']
    in_maps = marshal(inputs)
    res = run_bass_kernel_spmd(nc, in_maps, list(range(NC_)))
    return unmarshal(res.results)

